# revision 73
# baseline (speedup 1.0000x reference)
"""DKVMN Bass kernel v2 — scan-instruction formulation, 8-core data-parallel over batch.

Math (per batch b): M^0 = Mv0;  M^t = (1 - w_t e_t^T) * M^{t-1} + w_t a_t^T (elementwise),
reads_t = sum_v w_t[v] M^t[v,:]  (M pre-update at step t).

Each (b,v,k) element follows the scalar recurrence  m_t = q_t * m_{t-1} + r_t  with
q = 1 - w[b,t,v] e[b,t,k], r = w[b,t,v] a[b,t,k].  The DVE TensorTensorScan instruction
computes exactly this along the free axis; a zero-in-data0 "reset column" at each segment
start reloads M0 so one instruction handles many independent (v) segments.

Layout: partition p = ks*16 + b  (ks in 0..7, b in 0..15); 16 k-tiles, tile j covers
k = 8j + ks.  Free axis = (v, t): col v*200 + t.  Column v*200 is the reset column
(q=0, r=M0[v,k]), cols v*200+t (t>=1) hold q_{t-1}, r_{t-1}; scan output col v*200+t
is then M^t for t=0..199 — exactly the pre-update states the reads need.

reads[b,t,k] = sum_v w[b,t,v] * M^t: one elementwise multiply with w_rep (w replicated
across the 8 ks partition blocks) plus a strided tensor_reduce over v.

All layout changes go through DRAM scratch with >=400B contiguous runs; SBUF-side DMA
APs only ever use whole tiles or contiguous partition slices (partition dim leading).

Weights (embedding tables, projection matrices, Mv0) are baked into the NEFF as
Const tensors (uploaded once at model load); the only per-call input is the
question/correctness index block.  kernel() re-builds if the weights change.
"""

import numpy as np

import concourse.bacc as bacc
import concourse.bass as bass
import concourse.mybir as mybir
from concourse.tile import TileContext
from concourse.masks import make_identity

F32 = mybir.dt.float32
BF16 = mybir.dt.bfloat16
I32 = mybir.dt.int32
AX = mybir.AxisListType
ALU = mybir.AluOpType
ACTF = mybir.ActivationFunctionType

B, S, DK, DV, NQ = 128, 200, 128, 64, 10000
NC = 8
BL = B // NC          # 16 batches per core
L = BL * S            # 3200 lookups per core
NJ = L // 128         # 25 gather groups
CH = 400              # matmul free-dim chunk
NCH = L // CH         # 8 chunks

DT = BF16             # compute dtype for the scan phase
VCH = 32              # v per scan chunk
NVC = DV // VCH       # chunks per k-tile
CF = VCH * S          # free size of one scan chunk
NKT = DK // 8         # 16 k-tiles

# packed small-tensor layout: name -> (word_offset, words, dtype, partitions, free)
def _build_pack_layout():
    layout = {}
    off = 0
    def add(name, dtype, p, f):
        nonlocal off
        esz = 4 if dtype in (F32, I32) else 2
        words = (p * f * esz + 3) // 4
        layout[name] = (off, words, dtype, p, f)
        off += words
    add("qidx", I32, 128, NJ)
    add("xidx", I32, 128, NJ)
    add("M0r", DT, 128, NKT * DV)
    add("MkT", DT, DK, DV)
    add("eW", DT, DK, DK)
    add("aW", DT, DK, DK)
    add("fWr", DT, DK, DK)
    add("fWk", DT, DK, DK)
    add("pW", DT, DK, 1)
    add("eb", F32, DK, 1)
    add("ab", F32, DK, 1)
    add("fb", F32, DK, 1)
    add("pb", F32, 1, 1)
    return layout, off

PACK_LAYOUT, PACK_WORDS = _build_pack_layout()


def build_kernel(weights, mode="full"):
    nc = bacc.Bacc("TRN2", target_bir_lowering=False, debug=False, num_devices=NC)

    # ---- one per-call input (indices); everything else baked as Const ----
    idx = nc.dram_tensor("idx", [128, 2 * NJ], I32, kind="ExternalInput").ap()
    out = nc.dram_tensor("out", [1, L], F32, kind="ExternalOutput").ap()
    consts = {k: nc.inline_tensor(v, name=f"c_{k}").ap() for k, v in weights.items()}
    emb = consts.get("emb")

    def seg(name):
        return consts[name]
    w_dram2 = nc.dram_tensor("w2_scr", [BL, DV * S], DT).ap()
    # ea_scr col = b*2S + x*S + t  (x: 0=e, 1=a) so the per-j relayout load is
    # a 3-dim AP with contiguous 400-elem (x,t) runs
    ea_dram = nc.dram_tensor("ea_scr", [128, 2 * L], DT).ap()
    r_dram = nc.dram_tensor("r_scr", [128, L], DT).ap()

    with TileContext(nc) as tc:
        with (
            tc.tile_pool(name="persist", bufs=1) as pp,
            tc.tile_pool(name="work", bufs=2) as wp,
            tc.tile_pool(name="qb", bufs=2) as qpool,
            tc.tile_pool(name="rb", bufs=2) as rpool,
            tc.tile_pool(name="web", bufs=2) as wepool,
            tc.tile_pool(name="wab", bufs=2) as wapool,
            tc.tile_pool(name="mb", bufs=2) as mpool,
            tc.tile_pool(name="wmb", bufs=2) as wmpool,
            tc.tile_pool(name="tb", bufs=2) as tpool,
            tc.tile_pool(name="psum", bufs=2, space="PSUM") as pu,
            tc.tile_pool(name="psum_t", bufs=2, space="PSUM") as put,
        ):
            # ---------- params ----------
            # idx first: the gather stream is the critical path
            idx_sb = pp.tile([128, 2 * NJ], I32)
            nc.sync.dma_start(out=idx_sb[:], in_=idx)
            ident = pp.tile([128, 128], DT)
            make_identity(nc, ident[:])
            MkT_sb = pp.tile([DK, DV], DT)
            nc.sync.dma_start(out=MkT_sb[:], in_=seg("MkT"))
            eW_sb = pp.tile([DK, DK], DT)
            nc.sync.dma_start(out=eW_sb[:], in_=seg("eW"))
            aW_sb = pp.tile([DK, DK], DT)
            nc.sync.dma_start(out=aW_sb[:], in_=seg("aW"))
            fWr_sb = pp.tile([DK, DK], DT)
            nc.scalar.dma_start(out=fWr_sb[:], in_=seg("fWr"))
            fWk_sb = pp.tile([DK, DK], DT)
            nc.scalar.dma_start(out=fWk_sb[:], in_=seg("fWk"))
            pW_sb = pp.tile([DK, 1], DT)
            nc.scalar.dma_start(out=pW_sb[:], in_=seg("pW"))
            eb_sb = pp.tile([DK, 1], F32)
            nc.sync.dma_start(out=eb_sb[:], in_=seg("eb"))
            ab_sb = pp.tile([DK, 1], F32)
            nc.sync.dma_start(out=ab_sb[:], in_=seg("ab"))
            fb_sb = pp.tile([DK, 1], F32)
            nc.scalar.dma_start(out=fb_sb[:], in_=seg("fb"))
            pb_sb = pp.tile([1, 1], F32)
            nc.scalar.dma_start(out=pb_sb[:], in_=seg("pb"))
            M0_sb = pp.tile([128, NKT * DV], DT)
            nc.sync.dma_start(out=M0_sb[:], in_=seg("M0r"))


            k_t = pp.tile([128, L], DT)        # [kin, l]
            reads_all = pp.tile([128, L], DT)  # [(ks,b), (j,t)]
            partials = pp.tile([128, NVC * S], DT)

            prep_pool = tc.tile_pool(name="prep", bufs=1)
            prp = prep_pool.__enter__()
            v_t = prp.tile([128, L], DT)     # [kin, l]
            e_sb = prp.tile([128, L], DT)    # [kout, l]
            a_sb = prp.tile([128, L], DT)
            w_T = prp.tile([64, L], DT)      # [v, l]

            # ---------- gather + transpose (k first: w path is longest) ----------
            GW = 1  # gather batch width (multi-offset indirect DMA unsupported: one descriptor per partition)
            if mode == "empty":
                nc.vector.memset(k_t[:], 0.0)
                nc.vector.memset(v_t[:], 0.0)
                nc.vector.memset(e_sb[:], 0.0)
                nc.vector.memset(a_sb[:], 0.0)
                nc.vector.memset(w_T[:], 0.0)
            for j in range(NJ if mode != "empty" else 0):
                ksl = wp.tile([128, 128], DT, tag="gk", bufs=4)
                nc.gpsimd.indirect_dma_start(
                    out=ksl[:], out_offset=None, in_=emb,
                    in_offset=bass.IndirectOffsetOnAxis(ap=idx_sb[:, j : j + 1], axis=0),
                )
                tp = put.tile([128, 128], DT, tag="tr")
                nc.tensor.transpose(out=tp[:], in_=ksl[:], identity=ident[:])
                # PSUM->SBUF copies on vector: rd0-only (PSUM has one DVE read
                # port) so no shared-port conflict with SWDGE gather
                # descriptor-gen, and it unclogs scalar (the gather-rate gate)
                nc.vector.tensor_copy(out=k_t[:, j * 128 : (j + 1) * 128], in_=tp[:])

                # scores + softmax + transpose into w_T.  Logits here are
                # bounded (~|0.2|): skip the max-subtraction, exp directly.
                wps = pu.tile([128, DV], F32, tag="mm")
                nc.tensor.matmul(
                    out=wps[:], lhsT=k_t[:, j * 128 : (j + 1) * 128], rhs=MkT_sb[:],
                    start=True, stop=True,
                )
                expt = wp.tile([128, DV], DT, tag="expt", bufs=3)
                sums = wp.tile([128, 1], F32, tag="sums", bufs=3)
                nc.scalar.activation(
                    out=expt[:], in_=wps[:], func=ACTF.Exp, accum_out=sums[:],
                )
                rsum = wp.tile([128, 1], F32, tag="rsum", bufs=3)
                nc.vector.reciprocal(out=rsum[:], in_=sums[:])
                wblk = wp.tile([128, DV], DT, tag="wblk", bufs=3)
                # NOTE: this 1-src even-dim SBUF op runs in 4x_2P mode and
                # briefly holds the shared port (~240ns/j against SWDGE);
                # the Scalar Copy+scale-ptr alternative wedged the device
                # once (NRT_EXEC_UNIT_UNRECOVERABLE) — not worth the risk.
                nc.vector.tensor_scalar_mul(wblk[:], expt[:], rsum[:, :1])
                tpw = put.tile([64, 128], DT, tag="trw")
                nc.tensor.transpose(out=tpw[:], in_=wblk[:], identity=ident[:])
                nc.vector.tensor_copy(out=w_T[:, j * 128 : (j + 1) * 128], in_=tpw[:])

            for j in range(NJ if mode != "empty" else 0):
                vsl = wp.tile([128, 128], DT, tag="gv", bufs=4)
                nc.gpsimd.indirect_dma_start(
                    out=vsl[:], out_offset=None, in_=emb,
                    in_offset=bass.IndirectOffsetOnAxis(
                        ap=idx_sb[:, NJ + j : NJ + j + 1], axis=0
                    ),
                )
                tp2 = put.tile([128, 128], DT, tag="tr")
                nc.tensor.transpose(out=tp2[:], in_=vsl[:], identity=ident[:])
                nc.vector.tensor_copy(out=v_t[:, j * 128 : (j + 1) * 128], in_=tp2[:])

            # ---------- w relayout FIRST (only depends on the k-gathers, which
            # finish ~45us before the v-gathers): enqueue dump+loads on the
            # queues ahead of the e/a dumps so FIFO head-of-line blocking
            # doesn't delay them behind v-gather-dependent work ----------
            dma_engines = [nc.sync, nc.scalar]
            # w_rep[p=(ks,b), v*S+t] = w[b,t,v]; stage b-major in DRAM once so
            # the 8 replication loads are fully contiguous
            w_rep = pp.tile([128, DV * S], DT)
            nc.sync.dma_start(
                out=w_dram2.rearrange("b (v t) -> v b t", t=S),
                in_=w_T[:].rearrange("v (b t) -> v b t", t=S),
            )
            for ks in range(8):
                nc.sync.dma_start(
                    out=w_rep[16 * ks : 16 * ks + 16, :], in_=w_dram2,
                )

            # ---------- e = sigmoid(v@eW+eb), a = tanh(v@aW+ab): [kout, l] ----------
            for c in range(NCH):
                cs = slice(c * CH, (c + 1) * CH)
                eps = pu.tile([128, CH], F32, tag="mm")
                nc.tensor.matmul(out=eps[:], lhsT=eW_sb[:], rhs=v_t[:, cs], start=True, stop=True)
                nc.scalar.activation(out=e_sb[:, cs], in_=eps[:], func=ACTF.Sigmoid, bias=eb_sb[:, :1])
                nc.sync.dma_start(
                    out=ea_dram.rearrange("r (b x t) -> r b x t", x=2, t=S)[
                        :, 2 * c : 2 * c + 2, 0
                    ],
                    in_=e_sb[:, cs].rearrange("r (b t) -> r b t", t=S),
                )
                aps = pu.tile([128, CH], F32, tag="mm")
                nc.tensor.matmul(out=aps[:], lhsT=aW_sb[:], rhs=v_t[:, cs], start=True, stop=True)
                nc.scalar.activation(out=a_sb[:, cs], in_=aps[:], func=ACTF.Tanh, bias=ab_sb[:, :1])
                nc.scalar.dma_start(
                    out=ea_dram.rearrange("r (b x t) -> r b x t", x=2, t=S)[
                        :, 2 * c : 2 * c + 2, 1
                    ],
                    in_=a_sb[:, cs].rearrange("r (b t) -> r b t", t=S),
                )

            # ea_bk[j][p=(ks,b), x*S+t] = {e,a}[b,t,8j+ks] — ONE load per j
            # (e and a share one DRAM scratch) so the post-gather DMA-issue
            # cascade is 16 instructions, not 64, and scan iteration j only
            # waits for its own load.  First PRE_J up front, rest in-loop.
            PRE_J = 2
            ea_bk = [pp.tile([128, 2 * S], DT, name=f"ea_bk{j}") for j in range(NKT)]
            # col of ea_dram = b*2S + x*S + t; iterate in (ks, b, x, t) order to
            # match the out tile's flattened (p=(ks,b), (x,t)) element order;
            # x,t merge into one contiguous 400-elem run -> 3-dim DMA AP
            ea_view = ea_dram.rearrange("(j ks) (b x t) -> j ks b x t", ks=8, x=2, t=S)
            for j in range(PRE_J):
                # scalar queue: right behind the last a-dump, ahead of iter-0
                # q/r prep — completes ~3us after the matmul chunks finish
                nc.scalar.dma_start(out=ea_bk[j][:], in_=ea_view[j])

            prep_pool.__exit__(None, None, None)

            # m/wm first live after prep closes: this pool reuses the freed
            # prep space (v_t/e_sb/a_sb/w_T = 25.6KB) exactly
            scanw_pool = tc.tile_pool(name="scanw", bufs=1)
            swp = scanw_pool.__enter__()

            # ---------- scan phase ----------
            # Engine assignment is driven by the TRN2 shared-SBUF-port rule:
            # GpSimd and the DVE's 2nd read port share one exclusive lock, so
            # ANY GpSimd op fully blocks every 2-src vector op (TT / scan) for
            # its whole duration.  GpSimd therefore does nothing here; Vector
            # gets only flat unit-stride ops (2x perf mode); Scalar (own
            # ports) does the shifted copies.
            if mode != "full":
                nc.vector.memset(reads_all[:], 0.0)
            # reset columns of the q tiles never change: zero both pool
            # buffers once instead of a memset per iteration
            if mode == "full":
                for _ in range(2):
                    qz = qpool.tile([128, CF], DT, tag="q", name="qz")
                    nc.vector.memset(
                        qz[:].rearrange("p (v t) -> p v t", v=VCH)[:, :, :1], 0.0
                    )
            for j in range(NKT if mode == "full" else 0):
                if j + PRE_J < NKT:
                    nc.sync.dma_start(
                        out=ea_bk[j + PRE_J][:], in_=ea_view[j + PRE_J]
                    )
                for h in range(NVC):
                    vbase = h * VCH
                    wsl = w_rep[:, vbase * S : (vbase + VCH) * S]
                    # we = w * e, wa = w * a as SEPARATE TTs: a merged 6400-col
                    # op measured worse — scalar's q-prep can start right after
                    # the 1.7us we instead of a 3.4us fused op
                    wet = wepool.tile([128, CF], DT, tag="we")
                    nc.vector.tensor_tensor(
                        out=wet[:].rearrange("p (v t) -> p v t", v=VCH),
                        in0=wsl.rearrange("p (v t) -> p v t", v=VCH),
                        in1=ea_bk[j][:, :S]
                        .rearrange("p (u t) -> p u t", u=1)
                        .to_broadcast([128, VCH, S]),
                        op=ALU.mult,
                    )
                    wat = wapool.tile([128, CF], DT, tag="wa")
                    nc.vector.tensor_tensor(
                        out=wat[:].rearrange("p (v t) -> p v t", v=VCH),
                        in0=wsl.rearrange("p (v t) -> p v t", v=VCH),
                        in1=ea_bk[j][:, S:]
                        .rearrange("p (u t) -> p u t", u=1)
                        .to_broadcast([128, VCH, S]),
                        op=ALU.mult,
                    )
                    we = wet[:]
                    wa = wat[:]
                    # q: cols v*S (reset) = 0 (pre-zeroed); cols v*S+1+tau = 1 - we[tau]
                    q = qpool.tile([128, CF], DT, tag="q")
                    q3 = q[:].rearrange("p (v t) -> p v t", v=VCH)
                    nc.scalar.activation(
                        out=q3[:, :, 1:],
                        in_=we.rearrange("p (v t) -> p v t", v=VCH)[:, :, : S - 1],
                        func=ACTF.Copy, bias=1.0, scale=-1.0,
                    )
                    # r: cols v*S = M0; cols v*S+1+tau = wa[tau]
                    r = rpool.tile([128, CF], DT, tag="r")
                    r3 = r[:].rearrange("p (v t) -> p v t", v=VCH)
                    nc.scalar.copy(
                        out=r3[:, :, 0],
                        in_=M0_sb[:, j * DV + vbase : j * DV + vbase + VCH],
                    )
                    nc.scalar.copy(
                        out=r3[:, :, 1:],
                        in_=wa.rearrange("p (v t) -> p v t", v=VCH)[:, :, : S - 1],
                    )
                    # the scan: M[col v*S+t] = M^t
                    # m/wm are vector-produced AND vector-consumed (same
                    # engine, serial) — single-buffered to fit VCH=32 in SBUF
                    m = swp.tile([128, CF], DT, tag="m")
                    nc.vector.tensor_tensor_scan(
                        out=m[:], data0=q[:], data1=r[:], initial=0.0,
                        op0=ALU.mult, op1=ALU.add,
                    )
                    # wm = w * M ; partial reads = sum_v wm (flat TT add tree)
                    wm = swp.tile([128, CF], DT, tag="wm")
                    nc.vector.tensor_tensor(
                        out=wm[:], in0=wsl, in1=m[:], op=ALU.mult,
                    )
                    with nc.allow_low_precision(reason="reads in bf16 is fine at 2e-2 tol"):
                        # add tree fully in-place in wm (vector-only tile)
                        sz = CF
                        while sz > 2 * S:
                            nc.vector.tensor_tensor(
                                out=wm[:, : sz // 2], in0=wm[:, : sz // 2],
                                in1=wm[:, sz // 2 : sz], op=ALU.add,
                            )
                            sz //= 2
                        nc.vector.tensor_tensor(
                            out=partials[:, h * S : (h + 1) * S],
                            in0=wm[:, :S], in1=wm[:, S : 2 * S], op=ALU.add,
                        )
                # combine the NVC=2 partials -> reads_all cols j
                with nc.allow_low_precision(reason="reads in bf16 is fine at 2e-2 tol"):
                    nc.vector.tensor_tensor(
                        out=reads_all[:, j * S : (j + 1) * S],
                        in0=partials[:, :S], in1=partials[:, S:], op=ALU.add,
                    )
                nc.scalar.dma_start(
                    out=r_dram[:, j * S : (j + 1) * S],
                    in_=reads_all[:, j * S : (j + 1) * S],
                )

            scanw_pool.__exit__(None, None, None)

            # ---------- finish: reads relayout via DRAM + f/pred ----------
            reads_t = pp.tile([128, L], DT)   # [k, l]
            rd_view = r_dram.rearrange("(ks b) (jj t) -> jj ks b t", b=BL, t=S)
            for j in range(NKT):
                dma_engines[(j + 1) % 2].dma_start(
                    out=reads_t[8 * j : 8 * j + 8, :].rearrange("ks (b t) -> ks b t", t=S),
                    in_=rd_view[j],
                )

            pred = pp.tile([1, L], F32)
            for c in range(NCH):
                cs = slice(c * CH, (c + 1) * CH)
                fps = pu.tile([128, CH], F32, tag="mm")
                nc.tensor.matmul(out=fps[:], lhsT=fWr_sb[:], rhs=reads_t[:, cs], start=True, stop=False)
                nc.tensor.matmul(out=fps[:], lhsT=fWk_sb[:], rhs=k_t[:, cs], start=False, stop=True)
                f_sb = wp.tile([128, CH], DT, tag="fsb")
                nc.scalar.activation(out=f_sb[:], in_=fps[:], func=ACTF.Tanh, bias=fb_sb[:, :1])
                pps = pu.tile([1, CH], F32, tag="mmp")
                nc.tensor.matmul(out=pps[:], lhsT=pW_sb[:], rhs=f_sb[:], start=True, stop=True)
                nc.scalar.activation(out=pred[:, cs], in_=pps[:], func=ACTF.Sigmoid, bias=pb_sb[:, :1])

            nc.sync.dma_start(out=out, in_=pred[:])

    nc.compile()
    return nc


# ------------------------------------------------------------------
def make_weights(k_emb, v_emb, Mk, Mv0, fW, fb_, eW, eb_, aW, ab_, pW, pb_):
    npdt = mybir.dt.np(DT)
    emb = np.concatenate(
        [np.asarray(k_emb, np.float32), np.asarray(v_emb, np.float32)], axis=0
    ).astype(npdt)

    # M0r[p=(ks,b), j*DV + v] = Mv0[v, 8j+ks]  (b-independent)
    Mv0 = np.asarray(Mv0, np.float32)          # [DV, DK]
    m0 = Mv0.T.reshape(NKT, 8, DV)             # [j, ks, v]
    m0 = np.transpose(m0, (1, 0, 2)).reshape(8, 1, NKT * DV)
    M0r = np.broadcast_to(m0, (8, BL, NKT * DV)).reshape(128, NKT * DV)

    fW = np.asarray(fW, np.float32)
    return {
        "emb": np.ascontiguousarray(emb),
        "M0r": np.ascontiguousarray(M0r.astype(npdt)),
        "MkT": np.ascontiguousarray(np.asarray(Mk, np.float32).T.astype(npdt)),
        "eW": np.ascontiguousarray(np.asarray(eW, np.float32).astype(npdt)),
        "aW": np.ascontiguousarray(np.asarray(aW, np.float32).astype(npdt)),
        "fWr": np.ascontiguousarray(fW[:DK].astype(npdt)),
        "fWk": np.ascontiguousarray(fW[DK:].astype(npdt)),
        "pW": np.ascontiguousarray(np.asarray(pW, np.float32).reshape(DK, 1).astype(npdt)),
        "eb": np.ascontiguousarray(np.asarray(eb_, np.float32).reshape(DK, 1)),
        "ab": np.ascontiguousarray(np.asarray(ab_, np.float32).reshape(DK, 1)),
        "fb": np.ascontiguousarray(np.asarray(fb_, np.float32).reshape(DK, 1)),
        "pb": np.ascontiguousarray(np.asarray(pb_, np.float32).reshape(1, 1)),
    }


def make_in_maps(question_seq, correct_seq, *args):
    q = np.asarray(question_seq).astype(np.int64)
    c = np.asarray(correct_seq).astype(np.int64)
    x = NQ + q + NQ * c          # rows of v_emb inside the combined emb table

    in_maps = []
    for core in range(NC):
        bs = slice(core * BL, (core + 1) * BL)
        qf = q[bs].reshape(-1)   # l = b*S + s
        xf = x[bs].reshape(-1)
        qi = qf.reshape(NJ, 128).T.astype(np.int32)   # [p, j]
        xi = xf.reshape(NJ, 128).T.astype(np.int32)
        in_maps.append({"idx": np.ascontiguousarray(np.concatenate([qi, xi], axis=1))})
    return in_maps


_CACHED = None
_CACHED_KEY = None


def _weights_key(weights):
    import hashlib

    h = hashlib.sha256()
    for k in sorted(weights):
        h.update(k.encode())
        h.update(np.ascontiguousarray(weights[k]).tobytes())
    return h.hexdigest()


def _get_nc(weights):
    global _CACHED, _CACHED_KEY
    key = _weights_key(weights)
    if _CACHED is None or _CACHED_KEY != key:
        _CACHED = build_kernel(weights)
        _CACHED_KEY = key
    return _CACHED


_EXEC = None  # (nc, jitted_callable) — reuse the PJRT executable across calls


def _build_exec(nc):
    """One jitted 8-core SPMD callable for nc (run_bass_via_pjrt rebuilds the
    jax.jit closure every call, ~2.3s; this caches it so repeat kernel()
    calls cost only dispatch)."""
    import jax
    from jax.sharding import Mesh, PartitionSpec
    from jax.experimental.shard_map import shard_map
    from concourse import bass2jax
    import concourse.mybir as mybir

    bass2jax.install_neuronx_cc_hook()
    partition_name = nc.partition_id_tensor.name if nc.partition_id_tensor else None
    in_names, out_names, out_avals, zero_outs = [], [], [], []
    for alloc in nc.m.functions[0].allocations:
        if not isinstance(alloc, mybir.MemoryLocationSet):
            continue
        name = alloc.memorylocations[0].name
        if alloc.kind == "ExternalInput":
            if name != partition_name:
                in_names.append(name)
        elif alloc.kind == "ExternalOutput":
            shape = tuple(alloc.tensor_shape)
            dtype = mybir.dt.np(alloc.dtype)
            out_names.append(name)
            out_avals.append(jax.core.ShapedArray(shape, dtype))
            zero_outs.append(np.zeros(shape, dtype))
    n_params = len(in_names)
    all_names = in_names + out_names
    if partition_name is not None:
        all_names.append(partition_name)

    def _body(*args):
        operands = list(args)
        if partition_name is not None:
            operands.append(bass2jax.partition_id_tensor())
        outs = bass2jax._bass_exec_p.bind(
            *operands,
            out_avals=tuple(out_avals),
            in_names=tuple(all_names),
            out_names=tuple(out_names),
            lowering_input_output_aliases=(),
            sim_require_finite=True,
            sim_require_nnan=True,
            nc=nc,
        )
        return tuple(outs)

    devices = jax.devices()[:NC]
    mesh = Mesh(np.asarray(devices), ("core",))
    n_outs = len(out_avals)
    in_specs = (PartitionSpec("core"),) * (n_params + n_outs)
    out_specs = (PartitionSpec("core"),) * n_outs
    jitted = jax.jit(
        shard_map(_body, mesh=mesh, in_specs=in_specs, out_specs=out_specs,
                  check_rep=False),
        keep_unused=True,
    )
    concat_zero = [
        np.zeros((NC * z.shape[0], *z.shape[1:]), z.dtype) for z in zero_outs
    ]

    def run(in_maps):
        concat_in = [
            np.concatenate([np.asarray(m[nm]) for m in in_maps], axis=0)
            for nm in in_names
        ]
        out_arrs = jitted(*concat_in, *concat_zero)
        full = np.asarray(out_arrs[out_names.index("out")])
        return full.reshape(NC, *out_avals[out_names.index("out")].shape)

    return run


def kernel(**inputs):
    global _EXEC

    weights = make_weights(
        inputs["k_emb"], inputs["v_emb"], inputs["Mk"], inputs["Mv0"],
        inputs["fW"], inputs["fb"], inputs["eW"], inputs["eb"],
        inputs["aW"], inputs["ab"], inputs["pW"], inputs["pb"],
    )
    nc = _get_nc(weights)
    in_maps = make_in_maps(inputs["question_seq"], inputs["correct_seq"])
    try:
        if _EXEC is None or _EXEC[0] is not nc:
            _EXEC = (nc, _build_exec(nc))
        per_core = _EXEC[1](in_maps)
        outs = [per_core[c].reshape(BL, S) for c in range(NC)]
    except Exception:
        # fall back to the stock execution path
        from concourse.bass_utils import run_bass_kernel_spmd

        _EXEC = None
        res = run_bass_kernel_spmd(nc, in_maps, core_ids=list(range(NC)))
        outs = [r["out"].reshape(BL, S) for r in res.results]
    return np.concatenate(outs, axis=0).astype(np.float32)

